# revision 49
# baseline (speedup 1.0000x reference)
"""DeepSeek-style MoE block (SwiGLU experts, top-k routing) on 8 Trainium2 cores.

Expert-parallel sharding: each of the 8 cores owns E/8 = 2 experts and receives
only the tokens routed to those experts (host-side dispatch). The device kernel
computes, per expert e with gathered tokens XT [D, W] (transposed, W = slot
width):

    GT = W0e @ X^T            (PSUM, bf16 matmuls, DFF on partitions)
    UT = W1e @ X^T
    HT = coef * silu(s0*GT) * UT   (SBUF bf16, [DFF, W]; coef = s1*s2*cw
                                    pre-broadcast per token, cw = summed
                                    routing weights)
    Y^T = W2e @ HT            (PSUM, D on partitions, tokens free)

The host scatter-adds each expert's Y^T columns into the dense [T, D] output
(the unshard/combine step for expert-parallel sharding).

Perf notes (245us fp32r baseline -> ~146us; from NTFF traces):
 - bf16 everywhere halves HBM traffic vs fp32 (63 -> 31 MB/core); final rel
   err ~4.4e-3 vs the 2e-2 gate (fp8 e4m3 measures 6.3e-2 -- unusable).
   bf16 matmuls run 1 cycle/row at 2.4GHz at any width; LDWEIGHTS (one per
   matmul) hides under >=256-row moving operands.
 - Experts sorted by routed-token count into two slots (big/small); each
   slot's width = max over its 8 experts rounded to 8 (360/328 vs 384/384):
   -11% phase-1 moving work.
 - Two physical HWDGE rings (SP + Activation) carry all bulk DMAs,
   alternating; w2 rides them in program order (an SWDGE w2 prefetch at t=0
   starves the phase-1 ramp -- the Tile scheduler hoists gpsimd dma_starts
   regardless of emission order). s0/coef ride SWDGE: tiny and needed late.
 - DMA issues cost ~600ns of the fronting sequencer, and issues queued ahead
   of ACTIVATE stall the silu->ht chain: weights move as 4-k 512KB groups
   (per-k only for expert 0's first k-group, where the first matmul needs
   only k0's slices).
 - Phase-1 matmuls rotate across all 4 psum banks every k; long serial
   same-bank accumulation chains run 20-130% slower (RAW on the bank).
 - Phase-2 psum->sbuf copies go on DVE (idle in ph2); the scalar engine is
   busy fronting DMAs there and late copies block psum recycling.
 - DO NOT densify the PE schedule (e.g. warmup matmuls during the startup DMA
   wait): sustained dense activity trips a hardware throttle and the whole
   run drops to ~2.0GHz (186ns vs 155ns per 360-row matmul), a 15-30us net
   loss. The natural ~6% idle schedule holds 2.4GHz.
 - Host packs w01/w2/xt so every DMA is a plain dram slice with contiguous
   >=1KB partition lines.
"""

import os
import numpy as np

T, D, DFF, E, TOPK = 1024, 2048, 1024, 16, 6
NCORES, P = 8, 128
EPC = E // NCORES  # experts per core
KG = 4             # k-tiles per grouped w01 DMA

# Set by kernel() after each run: BassKernelResults (exec_time_ns when traced).
LAST_RESULT = None

_PROGRAM_CACHE = {}


def _build_program(widths, d=D, dff=DFF, use_silu=True):
    """Build + compile the SPMD single-core Bass program.

    widths: per-expert-slot token widths (W0, W1), each <= 512, multiple of 8.
    use_silu=False decomposes silu into sigmoid+mul (CoreSim lacks Silu).
    """
    import concourse.bacc as bacc
    import concourse.mybir as mybir
    import concourse.tile as tile

    f32 = mybir.dt.float32
    bf16 = mybir.dt.bfloat16
    Silu = mybir.ActivationFunctionType.Silu

    WX = max(widths)
    NTX = -(-WX // P)
    KD = d // P        # k-tiles over D (contraction of W0/W1 matmuls)
    KF = dff // P      # k-tiles over DFF (contraction of W2 matmul)
    DSW = min(512, d)  # output D slice width
    NDS = d // DSW     # output D slices
    FG = 2             # DFF f-tiles per PSUM group (psG/psU pairs)
    FGP = FG * P
    NFG = KF // FG
    NKG = KD // KG

    nc = bacc.Bacc("TRN2", target_bir_lowering=False, debug=False)

    xt_d = nc.dram_tensor("xt", [EPC, NKG, P, KG, WX], bf16,
                          kind="ExternalInput").ap()
    w01_d = nc.dram_tensor("w01", [EPC, NFG, NKG, P, KG, 2, FGP], bf16,
                           kind="ExternalInput").ap()
    w2_d = nc.dram_tensor("w2p", [EPC, NDS, 2, P, KF // 2, DSW], bf16,
                          kind="ExternalInput").ap()
    s0_d = nc.dram_tensor("s0v", [EPC, P, 1], f32, kind="ExternalInput").ap()
    coef_d = nc.dram_tensor("coefb", [EPC, P, WX], f32,
                            kind="ExternalInput").ap()
    y_d = nc.dram_tensor("yt", [EPC, d, WX], bf16, kind="ExternalOutput").ap()

    with tile.TileContext(nc) as tc:
        # Alternate large DMAs across the two physical HWDGE rings (SP +
        # Activation; DVE/PE cannot front HWDGE). DMA issues cost ~600ns of
        # the fronting engine's sequencer, and issues queued ahead of ACTIVATE
        # delay the silu -> ht chain and stall phase 2 -- hence 4-k grouped
        # transfers (4x fewer issues) rather than per-k.
        rings = [nc.sync, nc.scalar]
        ring_state = [0]

        def ring():
            ring_state[0] ^= 1
            return rings[ring_state[0]]

        with (
            tc.tile_pool(name="xt", bufs=2) as xt_pool,
            tc.tile_pool(name="w01", bufs=8) as w01_pool,
            tc.tile_pool(name="w2", bufs=4) as w2_pool,
            tc.tile_pool(name="ht", bufs=2) as ht_pool,
            tc.tile_pool(name="act", bufs=6) as act_pool,
            tc.tile_pool(name="out", bufs=3) as out_pool,
            tc.tile_pool(name="sc", bufs=8) as sc_pool,
            tc.tile_pool(name="pgu", bufs=6, space="PSUM") as pgu_pool,
            tc.tile_pool(name="py", bufs=2, space="PSUM") as py_pool,
        ):
            # NOTE: PE "warmup" matmuls during the startup DMA wait were tried
            # and REVERTED: dense artificial work trips the hardware activity
            # throttle and the whole run then executes at ~2.0GHz (186ns/
            # 360-row matmul) instead of 2.4GHz (155ns) -- a 13us net loss.
            # The natural ~6% idle of this schedule keeps the PE at full
            # clock; do not densify it artificially.
            #
            # Loop order is ph1(e0), ph1(e1), ph2(e0), ph2(e1): the w01
            # weight stream runs seamlessly across both experts (no mid-
            # kernel ph1->ph2->ph1 turns), and e0's w2 prefetches under the
            # whole of e1's ph1 instead of fighting the last f-group.
            hts = []
            for e in range(EPC):
                W = widths[e]
                # --- tiny per-expert inputs ride SWDGE, off the rings ---
                xt = xt_pool.tile([P, KD, W], bf16, tag="xt")
                s0_sb = sc_pool.tile([P, 1], f32, tag="s0")
                nc.gpsimd.dma_start(s0_sb[:], s0_d[e])
                # per-token combine weight, pre-broadcast to all partitions
                # host-side; folded into ht so phase 2 needs no scaling pass
                coefb = sc_pool.tile([P, WX], f32, tag="coefb")
                nc.gpsimd.dma_start(coefb[:, :W], coef_d[e, :, :W])

                # --- phase 1: HT = silu(s0 * W0 xT) * (W1 xT), [DFF, W] ---
                ht = ht_pool.tile([P, KF, W], bf16, tag="ht")
                for fg in range(NFG):
                    psG = [pgu_pool.tile([P, 512], f32, tag="pgu",
                                         name=f"psG_{e}_{fg}_{j}")
                           for j in range(FG)]
                    psU = [pgu_pool.tile([P, 512], f32, tag="pgu",
                                         name=f"psU_{e}_{fg}_{j}")
                           for j in range(FG)]
                    w01bs = []
                    for kg in range(NKG):
                        w01b = w01_pool.tile([P, KG, 2, FGP], bf16, tag="w01b")
                        w01bs.append(w01b)
                        if fg == 0:
                            if e == 0 and kg == 0:
                                # per-k transfers for the very first k-group:
                                # the first matmul needs only k0's slices, so
                                # don't make it wait on a full 4-k group.
                                # (Moving cold-start xt to SWDGE was tried:
                                # software-DGE delivery latency is ~9us --
                                # keep the critical path on the HWDGE rings.)
                                for kk in range(KG):
                                    ring().dma_start(xt[:, kg * KG + kk, :],
                                                     xt_d[e, kg, :, kk, :W])
                                    ring().dma_start(w01b[:, kk],
                                                     w01_d[e, fg, kg, :, kk])
                            else:
                                ring().dma_start(
                                    xt[:, kg * KG:(kg + 1) * KG, :],
                                    xt_d[e, kg, :, :, :W])
                                ring().dma_start(w01b[:], w01_d[e, fg, kg])
                        else:
                            ring().dma_start(w01b[:], w01_d[e, fg, kg])

                    def mm(ps, gu, j, k):
                        nc.tensor.matmul(
                            ps[:, :W],
                            w01bs[k // KG][:, k % KG, gu, j * P:(j + 1) * P],
                            xt[:, k, :W],
                            start=(k == 0), stop=(k == KD - 1))

                    def act(j):
                        f = fg * FG + j
                        sig = act_pool.tile([P, 512], f32, tag="sig")
                        ht_f = ht[:, f, :W]
                        if use_silu:
                            nc.scalar.activation(
                                sig[:, :W], psG[j][:, :W], Silu,
                                scale=s0_sb[:])
                        else:
                            nc.scalar.activation(
                                sig[:, :W], psG[j][:, :W],
                                mybir.ActivationFunctionType.Sigmoid,
                                scale=s0_sb[:])
                            nc.vector.tensor_mul(
                                sig[:, :W], sig[:, :W], psG[j][:, :W])
                        nc.vector.tensor_mul(
                            sig[:, :W], sig[:, :W], coefb[:, :W])
                        nc.vector.tensor_mul(
                            ht_f, sig[:, :W], psU[j][:, :W])

                    # rotate across all 4 psum banks every k: consecutive
                    # same-bank accumulating matmuls pay a RAW pipeline
                    # penalty (serial chains measured 187-368ns/matmul vs
                    # 156ns with 4-bank rotation)
                    for k in range(KD):
                        for j in range(FG):
                            mm(psG[j], 0, j, k)
                            mm(psU[j], 1, j, k)
                    for j in range(FG):
                        act(j)
                hts.append(ht)

            # --- phase 2: Y^T = W2 H, d on partitions, tokens free ---
            # (SWDGE prefetch of w2 was tried and reverted: the Tile
            # scheduler hoists gpsimd dma_starts to t~8us regardless of
            # any unrelated-op ordering, and 3MB of w2 on the wire
            # during the fg0 ramp starves the phase-1 weight stream.)
            for e in range(EPC):
                W = widths[e]
                ht = hts[e]
                for dsi in range(NDS):
                    w2b = w2_pool.tile([P, KF, DSW], bf16, tag="w2b")
                    for h in range(2):
                        ring().dma_start(
                            w2b[:, h * (KF // 2):(h + 1) * (KF // 2)],
                            w2_d[e, dsi, h])
                    NC = DSW // P
                    # the very last dsi flushes y in two halves so the
                    # kernel tail ends on a small transfer that overlaps
                    # the final chains
                    split_out = (e == EPC - 1 and dsi == NDS - 1)
                    ysb = out_pool.tile([P, NC, 512], bf16, tag="ysb")
                    for c in range(NC):
                        dblk = dsi * NC + c
                        psY = py_pool.tile([P, 512], f32, tag="py",
                                           name=f"psY_{e}_{dblk}")
                        for k in range(KF):
                            nc.tensor.matmul(
                                psY[:, :W],
                                w2b[:, k, c * P:(c + 1) * P],
                                ht[:, k, :W],
                                start=(k == 0), stop=(k == KF - 1))
                        # psum->sbuf (f32->bf16) on DVE: it is idle during
                        # ph2, while the scalar engine is busy fronting DMAs
                        nc.vector.tensor_copy(ysb[:, c, :W], psY[:, :W])
                        if split_out and c % 2 == 1:
                            c0 = c - 1
                            ring().dma_start(
                                y_d[e, (dsi * NC + c0) * P:
                                    (dsi * NC + c0 + 2) * P, :W]
                                .rearrange("(c p) t -> p c t", p=P),
                                ysb[:, c0:c0 + 2, :W])
                    if not split_out:
                        ring().dma_start(
                            y_d[e, dsi * DSW:(dsi + 1) * DSW, :W]
                            .rearrange("(c p) t -> p c t", p=P),
                            ysb[:, :, :W])

    nc.compile()
    return nc


def _prep_host(inputs):
    """Host-side dispatch: routing weights, per-expert token gather, layouts."""
    import ml_dtypes
    bf16 = ml_dtypes.bfloat16

    x = np.ascontiguousarray(np.asarray(inputs["x"], dtype=np.float32))
    w0 = np.asarray(inputs["w0"], dtype=np.float32)
    w1 = np.asarray(inputs["w1"], dtype=np.float32)
    w2 = np.asarray(inputs["w2"], dtype=np.float32)
    s0 = np.asarray(inputs["s0"], dtype=np.float32)
    s1 = np.asarray(inputs["s1"], dtype=np.float32)
    s2 = np.asarray(inputs["s2"], dtype=np.float32)
    se = np.asarray(inputs["selected_experts"]).astype(np.int64)
    rw = np.asarray(inputs["routing_weights"], dtype=np.float32)

    Tn, Dn = x.shape
    En, DFFn, _ = w0.shape
    KD = Dn // P
    KF = DFFn // P
    DSW = min(512, Dn)
    NDS = Dn // DSW
    FG = 2
    FGP = FG * P
    NFG = KF // FG
    NKG = KD // KG

    # combine weight per (expert, token): sum of routing weights over top-k
    cw = np.zeros((En, Tn), np.float32)
    cols = np.arange(Tn)
    for k in range(se.shape[1]):
        np.add.at(cw, (se[:, k], cols), rw[:, k])

    idx = [np.flatnonzero(cw[e] != 0.0) for e in range(En)]
    n = np.array([len(i) for i in idx])
    # sort experts by token count; slot 0 = 8 largest, slot 1 = 8 smallest.
    # Each slot's width = its max count rounded to 8 (>=256 not needed: bf16
    # matmuls run 1 cycle/row at any width).
    order = np.argsort(-n)
    slot_experts = [order[:NCORES], order[NCORES:]]
    widths = tuple(max(256, -(-int(n[s].max()) // 8) * 8)
                   for s in slot_experts)
    WX = max(widths)
    NTX = -(-WX // P)

    xT = np.ascontiguousarray(x.T)  # [D, T]
    in_maps = []
    for c in range(NCORES):
        xt = np.zeros((EPC, NKG, P, KG, WX), bf16)
        coef = np.zeros((EPC, P, WX), np.float32)
        s0v = np.zeros((EPC, P, 1), np.float32)
        w01 = np.empty((EPC, NFG, NKG, P, KG, 2, FGP), bf16)
        w2p = np.empty((EPC, NDS, 2, P, KF // 2, DSW), bf16)
        for j in range(EPC):
            e = int(slot_experts[j][c])
            ids = idx[e]
            # xt_p[kg, p, kk, t] = x^T[(kg*KG+kk)*P + p, ids[t]]
            xg = xT[:, ids].reshape(NKG, KG, P, len(ids)).transpose(0, 2, 1, 3)
            xt[j, :, :, :, :len(ids)] = xg.astype(bf16)
            coef[j, :, :len(ids)] = s1[e] * s2[e] * cw[e, ids]
            s0v[j, :, 0] = s0[e]
            # w01p[fg, kg, p, kk, {g,u}, f] = w{0,1}[e].T blocks
            a = w0[e].T.reshape(NKG, KG, P, NFG, FGP)
            b = w1[e].T.reshape(NKG, KG, P, NFG, FGP)
            gu = np.stack([a, b], axis=3)        # [NKG, KG, P, 2, NFG, FGP]
            w01[j] = gu.transpose(4, 0, 2, 1, 3, 5).astype(bf16)
            # w2p[dsi, half, p, kk, c] = w2[e].T[(half*KF/2+kk)*P+p, dsi*DSW+c]
            w2t = w2[e].T.reshape(2, KF // 2, P, NDS, DSW)
            w2p[j] = w2t.transpose(3, 0, 2, 1, 4).astype(bf16)
        in_maps.append({
            "xt": xt,
            "w01": w01,
            "w2p": w2p,
            "s0v": s0v,
            "coefb": coef,
        })
    return in_maps, idx, slot_experts, widths, (Tn, Dn, DFFn)


def _combine(results, idx, slot_experts, shapes):
    """Unshard: scatter-add per-expert outputs into the dense [T, D] output."""
    Tn, Dn, _ = shapes
    out = np.zeros((Tn, Dn), np.float32)
    for c in range(NCORES):
        yt = results[c]["yt"]
        for j in range(EPC):
            e = int(slot_experts[j][c])
            ids = idx[e]
            if len(ids):
                out[ids] += yt[j, :, :len(ids)].T.astype(np.float32)
    return out


def _ensure_axon_ntff_hook():
    """Provide antenv.axon_hooks if the image's antenv stub lacks it.

    concourse.bass_utils imports it unconditionally when BASS_TRACE/trace is
    set under axon; without this the run crashes. When libaxon_pjrt.so exposes
    the NRT-profile symbols we also install the real hook so NTFF profiling
    (HW exec times) works; otherwise tracing degrades to a warning.
    """
    import sys
    import types
    try:
        import antenv.axon_hooks  # noqa: F401
        return
    except ImportError:
        pass
    try:
        import antenv

        mod = types.ModuleType("antenv.axon_hooks")
        _state = {"hook": None}
        mod.set_axon_ntff_profile_hook = lambda h: _state.__setitem__("hook", h)
        mod.get_axon_ntff_profile_hook = lambda: _state["hook"]
        sys.modules["antenv.axon_hooks"] = mod
        antenv.axon_hooks = mod
        try:
            from trn_agent_boot.trn_boot import _ntff_profile_via_ctypes

            so = "/opt/axon/libaxon_pjrt.so"
            if os.path.exists(so):
                mod.set_axon_ntff_profile_hook(_ntff_profile_via_ctypes(so))
        except Exception:
            pass
    except Exception:
        pass


def kernel(**inputs) -> np.ndarray:
    global LAST_RESULT
    _ensure_axon_ntff_hook()
    from concourse.bass_utils import run_bass_kernel_spmd

    in_maps, idx, slot_experts, widths, shapes = _prep_host(inputs)

    key = widths + shapes
    nc = _PROGRAM_CACHE.get(key)
    if nc is None:
        nc = _build_program(widths, d=shapes[1], dff=shapes[2])
        _PROGRAM_CACHE[key] = nc

    res = run_bass_kernel_spmd(nc, in_maps, core_ids=list(range(NCORES)))
    LAST_RESULT = res
    return _combine(res.results, idx, slot_experts, shapes)
